# revision 1
# baseline (speedup 1.0000x reference)
"""FNet transformer block kernel for Trainium2 (8 NeuronCores, data-parallel over batch).

Math notes
----------
reference computes, per batch b:
    ft  = Re( FFT_seq( FFT_hid( FFT_hid( x ))))        (hidden FFT applied twice)
    u   = x + ft;  t = LayerNorm(u) * g + beta
    out = (gelu(t @ w1 + b1) @ w2 + b2) * mask

Double FFT along hidden (D=1024):  (F_D^2 x)[d] = D * x[(-d) mod D]  (real).
So with w[t, d] = 1024 * x[t, (-d) mod 1024]:
    ft = Re(F_S) @ w = C @ w,   C[s, t] = cos(2*pi*s*t/2048)   (S=2048)
C is symmetric in both index reflections: C[s,t] = C[2048-s,t] = C[s,2048-t].
Folding both halves turns the 2048x2048 cosine transform into a ~1025x1025 one:
    Z.T = wf.T @ Cf,  wf = t-folded w (1025 rows, padded to 1152),
    Cf[t,s] = C[t,s] for t,s in [0,1024] (padded to 1152x1056),
    ft.T[:, s] = Z.T[:, s] for s<=1024, else Z.T[:, 2048-s]  (free-dim mirror).

Everything downstream runs with activations transposed (d on partitions) until
FFN1, whose stationary operand is u.T, which flips the result back to natural
token-major layout; FFN2 flips again via PE transposes of H.

LayerNorm is applied through the FFN1 matmul:
    P[s,j] = r[s]*(A[s,j] - m[s]*wsum1[j]) + b1p[j],  A = u.T.T @ w1p
using two rank-1 (K=1) matmul updates into the PSUM accumulator and a
per-partition ACT scale r[s] fused into the GELU activation.
gamma/beta are folded into w1p/b1p on the host.
"""

import sys
from contextlib import ExitStack

import numpy as np

sys.path.insert(0, "/opt/trn_rl_repo")

import concourse.bass as bass  # noqa: E402
import concourse.mybir as mybir  # noqa: E402
import concourse.tile as tile
from concourse import bacc  # noqa: E402
from concourse.bass_utils import run_bass_kernel_spmd  # noqa: E402
from concourse.masks import make_identity  # noqa: E402

S, D = 2048, 1024
TF = 1152  # folded-t rows: 1025 padded up to 9*128
SF = 1056  # folded-s cols: 1025 padded up to 1056
NCORES = 8
LN_EPS = 1e-5
EPS_P = float(D) * float(D) * LN_EPS
F32 = mybir.dt.float32
F32R = mybir.dt.float32r
KT = TF // 128  # 9
DT = D // 128   # 8
BW = 256        # s-block width for the streaming phase
NB = S // BW    # 8
NCHUNKS = [(0, 512), (512, 512), (1024, 32)]  # FFT output column chunks of SF


def _r(ap):
    return ap.bitcast(F32R)


def _emit_kernel(ctx: ExitStack, tc: tile.TileContext, xT, wf, cf, w1p, w2,
                 wsum1r, b1pdr, b2r, onescol, onesrow, out):
    nc = tc.nc
    f32 = F32

    cpool = ctx.enter_context(tc.tile_pool(name="consts", bufs=1))
    ones_col = cpool.tile([128, 1], F32R, tag="ones_col")
    nc.sync.dma_start(ones_col[:], onescol[:])
    ones_row = cpool.tile([1, 128], F32R, tag="ones_row")
    nc.sync.dma_start(ones_row[:], onesrow[:])
    ones_11 = ones_row[0:1, 0:1]
    ident = cpool.tile([128, 128], f32, tag="ident")
    make_identity(nc, ident[:])
    eps_t = cpool.tile([1, 1], f32, tag="eps_t")
    nc.gpsimd.memset(eps_t[:], EPS_P)
    zero_col = cpool.tile([128, 1], f32, tag="zero_col")
    nc.gpsimd.memset(zero_col[:], 0.0)
    wsum1_s = cpool.tile([1, D], F32R, tag="wsum1")
    nc.sync.dma_start(wsum1_s[:], wsum1r[:])
    b1pd_s = cpool.tile([1, D], F32R, tag="b1pd")
    nc.sync.dma_start(b1pd_s[:], b1pdr[:])
    b2_s = cpool.tile([1, D], F32R, tag="b2")
    nc.sync.dma_start(b2_s[:], b2r[:])

    # w1 stays resident through the whole kernel
    w1pool = ctx.enter_context(tc.tile_pool(name="w1", bufs=1))
    w1_s = []
    for dt_ in range(DT):
        t_ = w1pool.tile([128, D], F32R, tag=f"w1_{dt_}")
        nc.sync.dma_start(t_[:], w1p[dt_ * 128:(dt_ + 1) * 128, :])
        w1_s.append(t_)

    # Z.T (folded FFT output), resident
    zpool = ctx.enter_context(tc.tile_pool(name="zt", bufs=1))
    zt_s = [zpool.tile([128, SF], f32, tag=f"zt{m}", name=f"zt{m}")
            for m in range(DT)]

    # ---------------- Phase 1: folded cosine transform ----------------
    with tc.tile_pool(name="fft_in", bufs=1) as fpool, \
         tc.tile_pool(name="fft_ps", bufs=4, space="PSUM") as fps:
        wf_s, cf_s = [], []
        for kt in range(KT):
            a = fpool.tile([128, D], F32R, tag=f"wf{kt}")
            nc.sync.dma_start(a[:], wf[kt * 128:(kt + 1) * 128, :])
            wf_s.append(a)
            c = fpool.tile([128, SF], F32R, tag=f"cf{kt}")
            nc.sync.dma_start(c[:], cf[kt * 128:(kt + 1) * 128, :])
            cf_s.append(c)
        for mt in range(DT):
            for (n0, nw) in NCHUNKS:
                ps = fps.tile([128, 512], f32, tag="fftps")
                for kt in range(KT):
                    nc.tensor.matmul(
                        ps[:, :nw],
                        _r(wf_s[kt][:, mt * 128:(mt + 1) * 128]),
                        _r(cf_s[kt][:, n0:n0 + nw]),
                        start=(kt == 0), stop=(kt == KT - 1),
                    )
                nc.vector.tensor_copy(zt_s[mt][:, n0:n0 + nw], ps[:, :nw])

    # w2 resident (allocated after the FFT pools release their SBUF)
    w2pool = ctx.enter_context(tc.tile_pool(name="w2", bufs=1))
    w2_s = []
    for jt in range(DT):
        t_ = w2pool.tile([128, D], F32R, tag=f"w2_{jt}")
        nc.sync.dma_start(t_[:], w2[jt * 128:(jt + 1) * 128, :])
        w2_s.append(t_)

    # ---------------- Phase 2: streamed residual+LN+FFN ----------------
    xpool = ctx.enter_context(tc.tile_pool(name="xt", bufs=9))
    upool = ctx.enter_context(tc.tile_pool(name="u", bufs=9))
    usqpool = ctx.enter_context(tc.tile_pool(name="usq", bufs=9))
    stpool = ctx.enter_context(tc.tile_pool(name="strow", bufs=2))
    hpool = ctx.enter_context(tc.tile_pool(name="h", bufs=2))
    htpool = ctx.enter_context(tc.tile_pool(name="ht", bufs=2))
    opool = ctx.enter_context(tc.tile_pool(name="o", bufs=2))
    ffps = ctx.enter_context(tc.tile_pool(name="ffps", bufs=1, space="PSUM"))
    ops_ = ctx.enter_context(tc.tile_pool(name="ops", bufs=1, space="PSUM"))
    htps = ctx.enter_context(tc.tile_pool(name="htps", bufs=2, space="PSUM"))
    stps = ctx.enter_context(tc.tile_pool(name="stps", bufs=1, space="PSUM"))
    svps = ctx.enter_context(tc.tile_pool(name="svps", bufs=1, space="PSUM"))

    for blk in range(NB):
        s0 = blk * BW
        # u.T block = x.T block + mirrored Z.T columns
        u_t = []
        xt_ts = []
        for dt_ in range(DT):
            xt_t = xpool.tile([128, BW], f32, tag="xt")
            xt_ts.append(xt_t)
            nc.sync.dma_start(xt_t[:], xT[dt_ * 128:(dt_ + 1) * 128, s0:s0 + BW])
            u = upool.tile([128, BW], F32R, tag="u")
            z = zt_s[dt_]
            if s0 + BW <= 1025:
                nc.vector.tensor_add(u[:], xt_t[:], z[:, s0:s0 + BW])
            elif s0 == 1024:
                nc.vector.tensor_add(u[:, 0:1], xt_t[:, 0:1], z[:, 1024:1025])
                nc.vector.tensor_add(u[:, 1:BW], xt_t[:, 1:BW],
                                     z[:, 1023:1024 - BW:-1])
            else:
                nc.vector.tensor_add(u[:], xt_t[:],
                                     z[:, 2048 - s0:2048 - s0 - BW:-1])
            u_t.append(u)

        # token stats via ones-matmuls: S1 and S2 side by side on partition 0
        # (matmul dst must start at partition 0)
        s12 = stps.tile([1, 2 * BW], f32, tag="s12")
        for dt_ in range(DT):
            nc.tensor.matmul(s12[0:1, 0:BW], _r(ones_col[:]), _r(u_t[dt_][:]),
                             start=(dt_ == 0), stop=(dt_ == DT - 1))
        usq_t = []
        for dt_ in range(DT):
            usq = usqpool.tile([128, BW], F32R, tag="usq")
            nc.vector.tensor_mul(usq[:], u_t[dt_][:], u_t[dt_][:])
            usq_t.append(usq)
        for dt_ in range(DT):
            nc.tensor.matmul(s12[0:1, BW:2 * BW], _r(ones_col[:]),
                             _r(usq_t[dt_][:]),
                             start=(dt_ == 0), stop=(dt_ == DT - 1))

        s1r = stpool.tile([1, BW], f32, tag="s1r")
        nc.vector.tensor_copy(s1r[:], s12[0:1, 0:BW])
        s2r = stpool.tile([1, BW], f32, tag="s2r")
        nc.vector.tensor_copy(s2r[:], s12[0:1, BW:2 * BW])
        sqr = stpool.tile([1, BW], f32, tag="sqr")
        nc.vector.tensor_mul(sqr[:], s1r[:], s1r[:])
        vr = stpool.tile([1, BW], f32, tag="vr")
        # vr = 1024*S2 - S1^2  (= 1024^2 * var)
        nc.vector.scalar_tensor_tensor(
            out=vr[:], in0=s2r[:], scalar=float(D), in1=sqr[:],
            op0=mybir.AluOpType.mult, op1=mybir.AluOpType.subtract)
        svr = stpool.tile([1, BW], F32R, tag="svr")
        # svr = sqrt(vr + 1024^2*eps) = 1024*sqrt(var+eps)
        nc.scalar.activation(svr[:], vr[:], mybir.ActivationFunctionType.Sqrt,
                             bias=eps_t[0:1, 0:1], scale=1.0)
        mnegr = stpool.tile([1, BW], F32R, tag="mnegr")
        nc.vector.tensor_scalar_mul(mnegr[:], s1r[:], -1.0 / float(D))

        for i in range(BW // 128):
            st = blk * (BW // 128) + i
            isl = slice(i * 128, (i + 1) * 128)
            # r column: reciprocal of sv, times 1024. Plain fp32 matmul —
            # N=1 violates fp32r ISA restrictions (and is cost-trivial).
            svc = svps.tile([128, 1], f32, tag="svc")
            nc.tensor.matmul(svc[:], svr[0:1, isl].bitcast(F32),
                             ones_11.bitcast(F32), start=True, stop=True)
            rcol = stpool.tile([128, 1], f32, tag="rcol")
            nc.vector.reciprocal(rcol[:], svc[:])
            rcol2 = stpool.tile([128, 1], f32, tag="rcol2")
            nc.vector.tensor_scalar_mul(rcol2[:], rcol[:], float(D))

            # FFN1: A = u.T.T @ w1p  (+ rank-1 LN corrections)
            psA = ffps.tile([128, D], f32, tag="psA")
            for dt_ in range(DT):
                for (c0, cw) in [(0, 512), (512, 512)]:
                    nc.tensor.matmul(psA[:, c0:c0 + cw],
                                     _r(u_t[dt_][:, isl]),
                                     _r(w1_s[dt_][:, c0:c0 + cw]),
                                     start=(dt_ == 0), stop=False)
            for (c0, cw) in [(0, 512), (512, 512)]:
                nc.tensor.matmul(psA[:, c0:c0 + cw], _r(mnegr[0:1, isl]),
                                 _r(wsum1_s[0:1, c0:c0 + cw]),
                                 start=False, stop=False)
                nc.tensor.matmul(psA[:, c0:c0 + cw], _r(svr[0:1, isl]),
                                 _r(b1pd_s[0:1, c0:c0 + cw]),
                                 start=False, stop=True)
            h_t = hpool.tile([128, D], f32, tag="h")
            nc.scalar.activation(h_t[:], psA[:],
                                 mybir.ActivationFunctionType.Gelu,
                                 bias=zero_col[:, 0:1], scale=rcol2[:, 0:1])

            # transpose H for FFN2
            htsb = htpool.tile([128, D], F32R, tag="htsb")
            for half in range(2):
                psHT = htps.tile([128, 512], f32, tag="psHT")
                for q in range(4):
                    jt = half * 4 + q
                    nc.tensor.transpose(psHT[:, q * 128:(q + 1) * 128],
                                        h_t[:, jt * 128:(jt + 1) * 128],
                                        ident[:])
                nc.vector.tensor_copy(htsb[:, half * 512:(half + 1) * 512],
                                      psHT[:])

            # FFN2: out = H @ w2 + b2
            psO = ops_.tile([128, D], f32, tag="psO")
            for jt in range(DT):
                for (c0, cw) in [(0, 512), (512, 512)]:
                    nc.tensor.matmul(psO[:, c0:c0 + cw],
                                     _r(htsb[:, jt * 128:(jt + 1) * 128]),
                                     _r(w2_s[jt][:, c0:c0 + cw]),
                                     start=(jt == 0), stop=False)
            for (c0, cw) in [(0, 512), (512, 512)]:
                nc.tensor.matmul(psO[:, c0:c0 + cw], _r(ones_row[:]),
                                 _r(b2_s[0:1, c0:c0 + cw]),
                                 start=False, stop=True)
            o_t = opool.tile([128, D], f32, tag="o")
            nc.scalar.copy(o_t[:], psO[:])
            nc.sync.dma_start(out[st * 128:(st + 1) * 128, :], o_t[:])


_NC_CACHE = {}


def _build_nc():
    if "nc" in _NC_CACHE:
        return _NC_CACHE["nc"]
    nc = bacc.Bacc("TRN2", target_bir_lowering=False, debug=False)
    xT = nc.declare_dram_parameter("xT", [D, S], F32, isOutput=False)
    wf = nc.declare_dram_parameter("wf", [TF, D], F32R, isOutput=False)
    cf = nc.declare_dram_parameter("cf", [TF, SF], F32R, isOutput=False)
    w1p = nc.declare_dram_parameter("w1p", [D, D], F32R, isOutput=False)
    w2 = nc.declare_dram_parameter("w2", [D, D], F32R, isOutput=False)
    wsum1r = nc.declare_dram_parameter("wsum1r", [1, D], F32R, isOutput=False)
    b1pdr = nc.declare_dram_parameter("b1pdr", [1, D], F32R, isOutput=False)
    b2r = nc.declare_dram_parameter("b2r", [1, D], F32R, isOutput=False)
    onescol = nc.declare_dram_parameter("onescol", [128, 1], F32R, isOutput=False)
    onesrow = nc.declare_dram_parameter("onesrow", [1, 128], F32R, isOutput=False)
    out = nc.declare_dram_parameter("out", [S, D], F32, isOutput=True)
    with tile.TileContext(nc) as tc:
        with ExitStack() as ctx:
            _emit_kernel(ctx, tc, xT, wf, cf, w1p, w2, wsum1r, b1pdr, b2r,
                         onescol, onesrow, out)
    nc.compile()
    _NC_CACHE["nc"] = nc
    return nc


def _host_prep(x, ln_g, ln_b, w1, b1, w2, b2):
    """Build per-core and shared device inputs (all float32)."""
    B = x.shape[0]
    # Folded cosine matrix
    tt = np.arange(1025, dtype=np.float64)
    Cf = np.zeros((TF, SF), np.float32)
    Cf[:1025, :1025] = np.cos(
        2.0 * np.pi * np.outer(tt, tt) / S).astype(np.float32)

    w1p = (w1 * ln_g[:, None]).astype(np.float32)
    b1p = (b1 + ln_b @ w1).astype(np.float32)
    wsum1 = w1p.sum(axis=0, dtype=np.float64).astype(np.float32).reshape(1, D)
    b1pd = (b1p / np.float32(D)).reshape(1, D)
    b2r = np.ascontiguousarray(b2.astype(np.float32).reshape(1, D))

    rev = np.concatenate([[0], np.arange(D - 1, 0, -1)])
    shared = dict(cf=Cf, w1p=w1p, w2=np.ascontiguousarray(w2, dtype=np.float32),
                  wsum1r=wsum1, b1pdr=b1pd, b2r=b2r,
                  onescol=np.ones((128, 1), np.float32),
                  onesrow=np.ones((1, 128), np.float32))

    in_maps = []
    for b in range(B):
        xb = np.asarray(x[b], np.float32)
        w = np.float32(D) * xb[:, rev]
        wf_ = np.zeros((TF, D), np.float32)
        wf_[0] = w[0]
        wf_[1024] = w[1024]
        wf_[1:1024] = w[1:1024] + w[2047:1024:-1]
        xT = np.ascontiguousarray(xb.T)
        in_maps.append(dict(xT=xT, wf=wf_, **shared))
    return in_maps


def _run(inputs, trace=False, trace_kwargs=None):
    x = np.asarray(inputs["x"], np.float32)
    in_maps = _host_prep(
        x,
        np.asarray(inputs["ln_g"], np.float32),
        np.asarray(inputs["ln_b"], np.float32),
        np.asarray(inputs["w1"], np.float32),
        np.asarray(inputs["b1"], np.float32),
        np.asarray(inputs["w2"], np.float32),
        np.asarray(inputs["b2"], np.float32),
    )
    nc = _build_nc()
    res = run_bass_kernel_spmd(nc, in_maps, list(range(NCORES)), trace=trace,
                               **(trace_kwargs or {}))
    outs = np.stack([np.asarray(res.results[b]["out"], np.float32)
                     for b in range(NCORES)])
    outs = outs * np.asarray(inputs["mask"], np.float32)
    return outs, res


def kernel(**inputs) -> np.ndarray:
    out, _ = _run(inputs, trace=False)
    return out



# revision 11
# speedup vs baseline: 1.2256x; 1.2256x over previous
"""FNet transformer block kernel for Trainium2 (8 NeuronCores, data-parallel over batch).

Math notes
----------
reference computes, per batch b:
    ft  = Re( FFT_seq( FFT_hid( FFT_hid( x ))))        (hidden FFT applied twice)
    u   = x + ft;  t = LayerNorm(u) * g + beta
    out = (gelu(t @ w1 + b1) @ w2 + b2) * mask

Double FFT along hidden (D=1024):  (F_D^2 x)[d] = D * x[(-d) mod D]  (real).
So with w[t, d] = 1024 * x[t, (-d) mod 1024]:
    ft = Re(F_S) @ w = C @ w,   C[s, t] = cos(2*pi*s*t/2048)   (S=2048)
C is symmetric in both index reflections: C[s,t] = C[2048-s,t] = C[s,2048-t].
Folding both halves turns the 2048x2048 cosine transform into a ~1025x1025 one:
    Z.T = wf.T @ Cf,  wf = t-folded w (1025 rows, padded to 1152),
    Cf[t,s] = C[t,s] for t,s in [0,1024],
    ft.T[:, s] = Z.T[:, s] for s<=1024, else Z.T[:, 2048-s]  (free-dim mirror).

All matmuls run in bf16 (PSUM accumulation stays fp32); the 2e-2 harness
tolerance leaves ample room (~3e-3 observed).

The whole pipeline keeps activations transposed (d/feat on partitions):
    FFN1 emits A.T directly:  A.T[j, s] = sum_d w1p[d, j] * uhat[d, s]
with uhat = u * R broadcast along partitions (R[s] = D / sqrt(vr[s]+eps'),
vr = D*S2 - S1^2), a K=1 rank-1 term (-S1/sv) x wsum1 for the mean
correction, and b1p applied as a per-partition ACT bias fused into the GELU.
FFN2 then uses H.T tiles as the stationary operand, giving out[s, j2] in
natural token-major layout with no PE transposes anywhere.

Stats for block b+1 are issued before FFN work of block b so the tensor
engine never waits on the LayerNorm reduction chain.
"""

import sys
from contextlib import ExitStack

import numpy as np
import ml_dtypes

sys.path.insert(0, "/opt/trn_rl_repo")

import concourse.bass as bass  # noqa: E402
import concourse.mybir as mybir  # noqa: E402
import concourse.tile as tile  # noqa: E402
from concourse import bacc  # noqa: E402
from concourse.bass_utils import run_bass_kernel_spmd  # noqa: E402

S, D = 2048, 1024
TF = 1152   # folded-t rows: 1025 padded up to 9*128
SFZ = 1032  # Z.T columns kept: 1025 padded up to 1032
NCORES = 8
LN_EPS = 1e-5
EPS_P = float(D) * float(D) * LN_EPS
F32 = mybir.dt.float32
F32R = mybir.dt.float32r
BF16 = mybir.dt.bfloat16
BF16NP = ml_dtypes.bfloat16
KT = TF // 128  # 9
DT = D // 128   # 8
BW = 512        # s-block width for the streaming phase
NB = S // BW    # 4
FCHUNKS = [(0, 512), (512, 512), (1024, 8)]  # FFT output column chunks
AF = mybir.ActivationFunctionType


def _emit_kernel(ctx: ExitStack, tc: tile.TileContext, xT, wf, cf, w1p, w2,
                 wsum1r, b1pc, b2r, onescol, onesrow, onesrowf, out):
    nc = tc.nc

    cpool = ctx.enter_context(tc.tile_pool(name="consts", bufs=1))
    ones_col = cpool.tile([128, 1], BF16, tag="ones_col")
    nc.sync.dma_start(ones_col[:], onescol[:])
    ones_row = cpool.tile([1, 128], BF16, tag="ones_row")
    nc.sync.dma_start(ones_row[:], onesrow[:])
    ones_rowf = cpool.tile([1, 128], F32R, tag="ones_rowf")
    nc.sync.dma_start(ones_rowf[:], onesrowf[:])
    wsum1_s = cpool.tile([1, D], BF16, tag="wsum1")
    nc.sync.dma_start(wsum1_s[:], wsum1r[:])
    b1p_s = cpool.tile([128, DT], F32, tag="b1p")
    nc.sync.dma_start(b1p_s[:], b1pc[:])
    b2_s = cpool.tile([1, D], BF16, tag="b2")
    nc.sync.dma_start(b2_s[:], b2r[:])
    eps_t = cpool.tile([1, 1], F32, tag="eps_t")
    nc.gpsimd.memset(eps_t[:], EPS_P)

    # FFT inputs (bf16): issued first so the tensor engine starts early
    fpool = ctx.enter_context(tc.tile_pool(name="fft_in", bufs=1))
    wf_s, cf_s = [], []
    for kt in range(KT):
        a = fpool.tile([128, D], BF16, tag=f"wf{kt}")
        nc.sync.dma_start(a[:], wf[kt * 128:(kt + 1) * 128, :])
        wf_s.append(a)
        c = fpool.tile([128, SFZ], BF16, tag=f"cf{kt}")
        nc.sync.dma_start(c[:], cf[kt * 128:(kt + 1) * 128, :])
        cf_s.append(c)

    # x.T prefetch: rotating pool, 2.5 blocks deep
    xpool = ctx.enter_context(tc.tile_pool(name="xt", bufs=20))
    xt_s = []
    for b in range(NB):
        row = []
        for dt_ in range(DT):
            t_ = xpool.tile([128, BW], BF16, tag="xt", name=f"x{b}_{dt_}")
            nc.sync.dma_start(
                t_[:], xT[dt_ * 128:(dt_ + 1) * 128, b * BW:(b + 1) * BW])
            row.append(t_)
        xt_s.append(row)

    w1pool = ctx.enter_context(tc.tile_pool(name="w1", bufs=1))
    w1_s = []
    for dt_ in range(DT):
        t_ = w1pool.tile([128, D], BF16, tag=f"w1_{dt_}")
        nc.sync.dma_start(t_[:], w1p[dt_ * 128:(dt_ + 1) * 128, :])
        w1_s.append(t_)
    w2pool = ctx.enter_context(tc.tile_pool(name="w2", bufs=1))
    w2_s = []
    for ft in range(DT):
        t_ = w2pool.tile([128, D], BF16, tag=f"w2_{ft}")
        nc.sync.dma_start(t_[:], w2[ft * 128:(ft + 1) * 128, :])
        w2_s.append(t_)

    # Z.T (folded FFT output), resident bf16
    zpool = ctx.enter_context(tc.tile_pool(name="zt", bufs=1))
    zt_s = [zpool.tile([128, SFZ], BF16, tag=f"zt{m}", name=f"zt{m}")
            for m in range(DT)]

    # phase-2 pools (SBUF)
    upool = ctx.enter_context(tc.tile_pool(name="u", bufs=16))
    usqpool = ctx.enter_context(tc.tile_pool(name="usq", bufs=9))
    ucpool = ctx.enter_context(tc.tile_pool(name="uc", bufs=9))
    stpool = ctx.enter_context(tc.tile_pool(name="strow", bufs=1))
    nrmpool = ctx.enter_context(tc.tile_pool(name="nrmrow", bufs=2))
    htpool = ctx.enter_context(tc.tile_pool(name="ht", bufs=9))
    opool = ctx.enter_context(tc.tile_pool(name="o", bufs=3))

    # stats PSUM pool lives through both phases (2 banks)
    stps = ctx.enter_context(tc.tile_pool(name="stps", bufs=1, space="PSUM"))

    def emit_stats(b):
        """residual add + square + column-sum matmuls for block b"""
        u_t, usq_t = [], []
        for dt_ in range(DT):
            u = upool.tile([128, BW], BF16, tag="u")
            z = zt_s[dt_]
            xt_t = xt_s[b][dt_]
            if b == 0:
                nc.vector.tensor_add(u[:], xt_t[:], z[:, 0:BW])
            elif b == 1:
                nc.vector.tensor_add(u[:], xt_t[:], z[:, BW:2 * BW])
            elif b == 2:
                nc.vector.tensor_add(u[:, 0:1], xt_t[:, 0:1], z[:, 1024:1025])
                nc.vector.tensor_add(u[:, 1:BW], xt_t[:, 1:BW],
                                     z[:, 1023:512:-1])
            else:
                nc.vector.tensor_add(u[:], xt_t[:], z[:, 512:0:-1])
            u_t.append(u)
        for dt_ in range(DT):
            usq = usqpool.tile([128, BW], BF16, tag="usq")
            nc.vector.tensor_mul(usq[:], u_t[dt_][:], u_t[dt_][:])
            usq_t.append(usq)
        s1 = stps.tile([1, BW], F32, tag="s1")
        for dt_ in range(DT):
            nc.tensor.matmul(s1[:], ones_col[:], u_t[dt_][:],
                             start=(dt_ == 0), stop=(dt_ == DT - 1))
        s2 = stps.tile([1, BW], F32, tag="s2")
        for dt_ in range(DT):
            nc.tensor.matmul(s2[:], ones_col[:], usq_t[dt_][:],
                             start=(dt_ == 0), stop=(dt_ == DT - 1))
        return u_t, s1, s2

    # ---------------- Phase 1: folded cosine transform ----------------
    pending = {}
    with tc.tile_pool(name="fft_ps", bufs=4, space="PSUM") as fps:
        for ci, (n0, nw) in enumerate(FCHUNKS):
            for mt in range(DT):
                ps = fps.tile([128, 512], F32, tag="fftps")
                for kt in range(KT):
                    nc.tensor.matmul(
                        ps[:, :nw],
                        wf_s[kt][:, mt * 128:(mt + 1) * 128],
                        cf_s[kt][:, n0:n0 + nw],
                        start=(kt == 0), stop=(kt == KT - 1),
                    )
                nc.scalar.copy(zt_s[mt][:, n0:n0 + nw], ps[:, :nw])
            if ci == 1:
                # block-0 stats slot in here: their adds only need chunk 0,
                # and the PE chews chunk 2 while the vector r-chain runs
                pending[0] = emit_stats(0)

    # ---------------- Phase 2: streamed LN+FFN, software-pipelined ----
    rbps = ctx.enter_context(tc.tile_pool(name="rbps", bufs=1, space="PSUM"))
    ffps = ctx.enter_context(tc.tile_pool(name="ffps", bufs=2, space="PSUM"))
    ops_ = ctx.enter_context(tc.tile_pool(name="ops", bufs=2, space="PSUM"))

    for b in range(NB):
        u_t, s1, s2 = pending.pop(b)

        # r-chain for block b (vector + one scalar sqrt; runs during the
        # previous block's FFN on the PE)
        s1r = stpool.tile([1, BW], F32, tag="s1r")
        nc.vector.tensor_copy(s1r[:], s1[:])
        s2r = stpool.tile([1, BW], F32, tag="s2r")
        nc.vector.tensor_copy(s2r[:], s2[:])
        sqr = stpool.tile([1, BW], F32, tag="sqr")
        nc.vector.tensor_mul(sqr[:], s1r[:], s1r[:])
        vr = stpool.tile([1, BW], F32, tag="vr")
        # vr = D*S2 - S1^2  (= D^2 * var)
        nc.vector.scalar_tensor_tensor(
            out=vr[:], in0=s2r[:], scalar=float(D), in1=sqr[:],
            op0=mybir.AluOpType.mult, op1=mybir.AluOpType.subtract)
        sv = stpool.tile([1, BW], F32, tag="sv")
        # sv = sqrt(vr + D^2*eps) = D*sqrt(var+eps)
        nc.scalar.activation(sv[:], vr[:], AF.Sqrt,
                             bias=eps_t[0:1, 0:1], scale=1.0)
        isv = stpool.tile([1, BW], F32, tag="isv")
        nc.vector.reciprocal(isv[:], sv[:])
        rrow = stpool.tile([1, BW], F32R, tag="rrow")
        nc.vector.tensor_scalar_mul(rrow[:], isv[:], float(D))
        nrm = nrmpool.tile([1, BW], BF16, tag="nrm")
        # nrm = -S1/sv  (rank-1 mean-correction coefficient row)
        nc.vector.scalar_tensor_tensor(
            out=nrm[:], in0=s1r[:], scalar=-1.0, in1=isv[:],
            op0=mybir.AluOpType.mult, op1=mybir.AluOpType.mult)
        # broadcast R down all 128 partitions (f32r matmul keeps it exact)
        rb = rbps.tile([128, BW], F32, tag="rb")
        nc.tensor.matmul(rb[:], ones_rowf[:], rrow[:], start=True, stop=True)

        if b + 1 < NB:
            pending[b + 1] = emit_stats(b + 1)

        # uhat = u * R (bf16 out)
        uc_t = []
        for dt_ in range(DT):
            uc = ucpool.tile([128, BW], BF16, tag="uc")
            nc.vector.tensor_mul(uc[:], u_t[dt_][:], rb[:])
            uc_t.append(uc)

        # FFN1 (transposed out): A.T[jf, s] accumulated per 128-feat tile
        ht_t = []
        for ft in range(DT):
            psA = ffps.tile([128, BW], F32, tag="psA")
            for kt in range(DT):
                nc.tensor.matmul(psA[:],
                                 w1_s[kt][:, ft * 128:(ft + 1) * 128],
                                 uc_t[kt][:],
                                 start=(kt == 0), stop=False)
            nc.tensor.matmul(psA[:], wsum1_s[0:1, ft * 128:(ft + 1) * 128],
                             nrm[:], start=False, stop=True)
            ht = htpool.tile([128, BW], BF16, tag="ht")
            nc.scalar.activation(ht[:], psA[:], AF.Gelu,
                                 bias=b1p_s[:, ft:ft + 1], scale=1.0)
            ht_t.append(ht)

        # FFN2: out[s, j2] with H.T tiles stationary
        for tt in range(BW // 128):
            o_t = opool.tile([128, D], F32, tag="o")
            for c in range(2):
                psO = ops_.tile([128, 512], F32, tag="psO")
                for ft in range(DT):
                    nc.tensor.matmul(psO[:],
                                     ht_t[ft][:, tt * 128:(tt + 1) * 128],
                                     w2_s[ft][:, c * 512:(c + 1) * 512],
                                     start=(ft == 0), stop=False)
                nc.tensor.matmul(psO[:], ones_row[:],
                                 b2_s[0:1, c * 512:(c + 1) * 512],
                                 start=False, stop=True)
                nc.scalar.copy(o_t[:, c * 512:(c + 1) * 512], psO[:])
            st = b * (BW // 128) + tt
            nc.sync.dma_start(out[st * 128:(st + 1) * 128, :], o_t[:])


_NC_CACHE = {}


def _build_nc():
    if "nc" in _NC_CACHE:
        return _NC_CACHE["nc"]
    nc = bacc.Bacc("TRN2", target_bir_lowering=False, debug=False)
    xT = nc.declare_dram_parameter("xT", [D, S], BF16, isOutput=False)
    wf = nc.declare_dram_parameter("wf", [TF, D], BF16, isOutput=False)
    cf = nc.declare_dram_parameter("cf", [TF, SFZ], BF16, isOutput=False)
    w1p = nc.declare_dram_parameter("w1p", [D, D], BF16, isOutput=False)
    w2 = nc.declare_dram_parameter("w2", [D, D], BF16, isOutput=False)
    wsum1r = nc.declare_dram_parameter("wsum1r", [1, D], BF16, isOutput=False)
    b1pc = nc.declare_dram_parameter("b1pc", [128, DT], F32, isOutput=False)
    b2r = nc.declare_dram_parameter("b2r", [1, D], BF16, isOutput=False)
    onescol = nc.declare_dram_parameter("onescol", [128, 1], BF16,
                                        isOutput=False)
    onesrow = nc.declare_dram_parameter("onesrow", [1, 128], BF16,
                                        isOutput=False)
    onesrowf = nc.declare_dram_parameter("onesrowf", [1, 128], F32R,
                                         isOutput=False)
    out = nc.declare_dram_parameter("out", [S, D], F32, isOutput=True)
    with tile.TileContext(nc) as tc:
        with ExitStack() as ctx:
            _emit_kernel(ctx, tc, xT, wf, cf, w1p, w2, wsum1r, b1pc, b2r,
                         onescol, onesrow, onesrowf, out)
    nc.compile()
    _NC_CACHE["nc"] = nc
    return nc


def _host_prep(x, ln_g, ln_b, w1, b1, w2, b2):
    """Build per-core and shared device inputs."""
    B = x.shape[0]
    # Folded cosine matrix (bf16)
    tt = np.arange(1025, dtype=np.float64)
    Cf = np.zeros((TF, SFZ), BF16NP)
    Cf[:1025, :1025] = np.cos(
        2.0 * np.pi * np.outer(tt, tt) / S).astype(BF16NP)

    w1p = (w1 * ln_g[:, None]).astype(np.float32)
    b1p = (b1 + ln_b @ w1).astype(np.float32)
    wsum1 = w1p.sum(axis=0, dtype=np.float64).astype(BF16NP).reshape(1, D)
    b1pc = np.ascontiguousarray(b1p.reshape(DT, 128).T)  # [128, 8] f32 cols
    b2r = b2.astype(BF16NP).reshape(1, D)

    rev = np.concatenate([[0], np.arange(D - 1, 0, -1)])
    shared = dict(cf=Cf, w1p=w1p.astype(BF16NP),
                  w2=np.ascontiguousarray(w2).astype(BF16NP),
                  wsum1r=wsum1, b1pc=b1pc, b2r=b2r,
                  onescol=np.ones((128, 1), BF16NP),
                  onesrow=np.ones((1, 128), BF16NP),
                  onesrowf=np.ones((1, 128), np.float32))

    in_maps = []
    for b in range(B):
        xb = np.asarray(x[b], np.float32)
        w = np.float32(D) * xb[:, rev]
        wf_ = np.zeros((TF, D), np.float32)
        wf_[0] = w[0]
        wf_[1024] = w[1024]
        wf_[1:1024] = w[1:1024] + w[2047:1024:-1]
        xT = np.ascontiguousarray(xb.T).astype(BF16NP)
        in_maps.append(dict(xT=xT, wf=wf_.astype(BF16NP), **shared))
    return in_maps


def _run(inputs, trace=False, trace_kwargs=None):
    x = np.asarray(inputs["x"], np.float32)
    in_maps = _host_prep(
        x,
        np.asarray(inputs["ln_g"], np.float32),
        np.asarray(inputs["ln_b"], np.float32),
        np.asarray(inputs["w1"], np.float32),
        np.asarray(inputs["b1"], np.float32),
        np.asarray(inputs["w2"], np.float32),
        np.asarray(inputs["b2"], np.float32),
    )
    nc = _build_nc()
    res = run_bass_kernel_spmd(nc, in_maps, list(range(NCORES)), trace=trace,
                               **(trace_kwargs or {}))
    outs = np.stack([np.asarray(res.results[b]["out"], np.float32)
                     for b in range(NCORES)])
    outs = outs * np.asarray(inputs["mask"], np.float32)
    return outs, res


def kernel(**inputs) -> np.ndarray:
    out, _ = _run(inputs, trace=False)
    return out


# revision 21
# speedup vs baseline: 1.5454x; 1.2610x over previous
"""FNet transformer block kernel for Trainium2 (8 NeuronCores, data-parallel over batch).

Math notes
----------
reference computes, per batch b:
    ft  = Re( FFT_seq( FFT_hid( FFT_hid( x ))))        (hidden FFT applied twice)
    u   = x + ft;  t = LayerNorm(u) * g + beta
    out = (gelu(t @ w1 + b1) @ w2 + b2) * mask

Double FFT along hidden (D=1024):  (F_D^2 x)[d] = D * x[(-d) mod D]  (real).
So with w[t, d] = 1024 * x[t, (-d) mod 1024]:
    ft = Re(F_S) @ w = C @ w,   C[s, t] = cos(2*pi*s*t/2048)   (S=2048)
C is symmetric in both index reflections: C[s,t] = C[2048-s,t] = C[s,2048-t].
Folding both halves turns the 2048x2048 cosine transform into a ~1025x1025 one:
    Z.T = wf.T @ Cf,  wf = t-folded w (1025 rows, padded to 1152),
    Cf[t,s] = C[t,s] for t,s in [0,1024],
    ft.T[:, s] = Z.T[:, s] for s<=1024, else Z.T[:, 2048-s]  (free-dim mirror).

All matmuls run in bf16 (PSUM accumulation stays fp32); the 2e-2 harness
tolerance leaves ample room (~3e-3 observed).

The whole pipeline keeps activations transposed (d/feat on partitions):
    FFN1 emits A.T directly:  A.T[j, s] = sum_d w1p[d, j] * uhat[d, s]
with uhat = u * R broadcast along partitions (R[s] = D / sqrt(vr[s]+eps'),
vr = D*S2 - S1^2), a K=1 rank-1 term (-S1/sv) x wsum1 for the mean
correction, and b1p applied as a per-partition ACT bias fused into the GELU.
FFN2 then uses H.T tiles as the stationary operand, giving out[s, j2] in
natural token-major layout with no PE transposes anywhere.

Stats for block b+1 are issued before FFN work of block b so the tensor
engine never waits on the LayerNorm reduction chain.
"""

import sys
from contextlib import ExitStack

import numpy as np
import ml_dtypes

sys.path.insert(0, "/opt/trn_rl_repo")

import concourse.bass as bass  # noqa: E402
import concourse.mybir as mybir  # noqa: E402
import concourse.tile as tile  # noqa: E402
from concourse import bacc  # noqa: E402
from concourse.bass_utils import run_bass_kernel_spmd  # noqa: E402

S, D = 2048, 1024
TF = 1152   # folded-t rows: 1025 padded up to 9*128
SFZ = 1032  # Z.T columns kept: 1025 padded up to 1032
NCORES = 8
LN_EPS = 1e-5
EPS_P = float(D) * float(D) * LN_EPS
F32 = mybir.dt.float32
F32R = mybir.dt.float32r
BF16 = mybir.dt.bfloat16
BF16NP = ml_dtypes.bfloat16
KT = TF // 128  # 9
DT = D // 128   # 8
BW = 512        # s-block width for the streaming phase
NB = S // BW    # 4
FCHUNKS = [(0, 512), (512, 512), (1024, 8)]  # FFT output column chunks
AF = mybir.ActivationFunctionType


def _emit_kernel(ctx: ExitStack, tc: tile.TileContext, xT, wf, cf, w1p, w2,
                 wsum1r, b1pc, b2rep, onescol, onesrowf, out):
    nc = tc.nc

    cpool = ctx.enter_context(tc.tile_pool(name="consts", bufs=1))
    ones_col = cpool.tile([128, 1], BF16, tag="ones_col")
    nc.sync.dma_start(ones_col[:], onescol[:])
    ones_rowf = cpool.tile([1, 128], F32R, tag="ones_rowf")
    nc.sync.dma_start(ones_rowf[:], onesrowf[:])
    wsum1_s = cpool.tile([1, D], BF16, tag="wsum1")
    nc.sync.dma_start(wsum1_s[:], wsum1r[:])
    b1p_s = cpool.tile([128, DT], F32, tag="b1p")
    nc.sync.dma_start(b1p_s[:], b1pc[:])
    b2rep_s = cpool.tile([128, D], F32, tag="b2rep")
    nc.sync.dma_start(b2rep_s[:], b2rep[:])
    eps_t = cpool.tile([1, 1], F32, tag="eps_t")
    nc.gpsimd.memset(eps_t[:], EPS_P)

    # FFT inputs (bf16): issued first so the tensor engine starts early
    fpool = ctx.enter_context(tc.tile_pool(name="fft_in", bufs=1))
    wf_s, cf_s = [], []
    for kt in range(KT):
        a = fpool.tile([128, D], BF16, tag=f"wf{kt}")
        nsplit = 4 if kt < 2 else 1
        cw = D // nsplit
        for i in range(nsplit):
            nc.sync.dma_start(a[:, i * cw:(i + 1) * cw],
                              wf[kt * 128:(kt + 1) * 128, i * cw:(i + 1) * cw])
        wf_s.append(a)
        c = fpool.tile([128, SFZ], BF16, tag=f"cf{kt}")
        if kt < 2:
            for i in range(4):
                c0, c1 = i * 258, min(SFZ, (i + 1) * 258)
                nc.sync.dma_start(c[:, c0:c1],
                                  cf[kt * 128:(kt + 1) * 128, c0:c1])
        else:
            nc.sync.dma_start(c[:], cf[kt * 128:(kt + 1) * 128, :])
        cf_s.append(c)

    # x.T prefetch: rotating pool, 2.5 blocks deep
    xpool = ctx.enter_context(tc.tile_pool(name="xt", bufs=20))
    xt_s = []
    for b in range(NB):
        row = []
        for dt_ in range(DT):
            t_ = xpool.tile([128, BW], BF16, tag="xt", name=f"x{b}_{dt_}")
            nc.sync.dma_start(
                t_[:], xT[dt_ * 128:(dt_ + 1) * 128, b * BW:(b + 1) * BW])
            row.append(t_)
        xt_s.append(row)

    w1pool = ctx.enter_context(tc.tile_pool(name="w1", bufs=1))
    w1_s = []
    for dt_ in range(DT):
        t_ = w1pool.tile([128, D], BF16, tag=f"w1_{dt_}")
        nc.sync.dma_start(t_[:], w1p[dt_ * 128:(dt_ + 1) * 128, :])
        w1_s.append(t_)
    w2pool = ctx.enter_context(tc.tile_pool(name="w2", bufs=1))
    w2_s = []
    for ft in range(DT):
        t_ = w2pool.tile([128, D], BF16, tag=f"w2_{ft}")
        nc.sync.dma_start(t_[:], w2[ft * 128:(ft + 1) * 128, :])
        w2_s.append(t_)

    # Z.T (folded FFT output), resident bf16
    zpool = ctx.enter_context(tc.tile_pool(name="zt", bufs=1))
    zt_s = [zpool.tile([128, SFZ], BF16, tag=f"zt{m}", name=f"zt{m}")
            for m in range(DT)]

    # phase-2 pools (SBUF)
    upool = ctx.enter_context(tc.tile_pool(name="u", bufs=16))
    usqpool = ctx.enter_context(tc.tile_pool(name="usq", bufs=9))
    ucpool = ctx.enter_context(tc.tile_pool(name="uc", bufs=9))
    stpool = ctx.enter_context(tc.tile_pool(name="strow", bufs=2))
    nrmpool = ctx.enter_context(tc.tile_pool(name="nrmrow", bufs=2))
    htpool = ctx.enter_context(tc.tile_pool(name="ht", bufs=9))
    opool = ctx.enter_context(tc.tile_pool(name="o", bufs=3))

    # stats + r-broadcast PSUM pools live through both phases (4 banks)
    stps = ctx.enter_context(tc.tile_pool(name="stps", bufs=1, space="PSUM"))
    rbps = ctx.enter_context(tc.tile_pool(name="rbps", bufs=2, space="PSUM"))

    def emit_stats(b):
        """residual add + square + column-sum matmuls for block b"""
        u_t, usq_t = [], []
        for dt_ in range(DT):
            u = upool.tile([128, BW], BF16, tag="u")
            z = zt_s[dt_]
            xt_t = xt_s[b][dt_]
            if b == 0:
                nc.vector.tensor_add(u[:], xt_t[:], z[:, 0:BW])
            elif b == 1:
                nc.vector.tensor_add(u[:], xt_t[:], z[:, BW:2 * BW])
            elif b == 2:
                nc.vector.tensor_add(u[:, 0:1], xt_t[:, 0:1], z[:, 1024:1025])
                nc.vector.tensor_add(u[:, 1:BW], xt_t[:, 1:BW],
                                     z[:, 1023:512:-1])
            else:
                nc.vector.tensor_add(u[:], xt_t[:], z[:, 512:0:-1])
            u_t.append(u)
        for dt_ in range(DT):
            usq = usqpool.tile([128, BW], BF16, tag="usq")
            nc.vector.tensor_mul(usq[:], u_t[dt_][:], u_t[dt_][:])
            usq_t.append(usq)
        s1 = stps.tile([1, BW], F32, tag="s1")
        for dt_ in range(DT):
            nc.tensor.matmul(s1[:], ones_col[:], u_t[dt_][:],
                             start=(dt_ == 0), stop=(dt_ == DT - 1))
        s2 = stps.tile([1, BW], F32, tag="s2")
        for dt_ in range(DT):
            nc.tensor.matmul(s2[:], ones_col[:], usq_t[dt_][:],
                             start=(dt_ == 0), stop=(dt_ == DT - 1))
        return u_t, s1, s2

    def emit_rchain(s1, s2):
        """LayerNorm reduction math: runs a full block ahead of its use"""
        s1r = stpool.tile([1, BW], F32, tag="s1r")
        nc.vector.tensor_copy(s1r[:], s1[:])
        s2r = stpool.tile([1, BW], F32, tag="s2r")
        nc.vector.tensor_copy(s2r[:], s2[:])
        sqr = stpool.tile([1, BW], F32, tag="sqr")
        nc.vector.tensor_mul(sqr[:], s1r[:], s1r[:])
        vr = stpool.tile([1, BW], F32, tag="vr")
        # vr = D*S2 - S1^2  (= D^2 * var)
        nc.vector.scalar_tensor_tensor(
            out=vr[:], in0=s2r[:], scalar=float(D), in1=sqr[:],
            op0=mybir.AluOpType.mult, op1=mybir.AluOpType.subtract)
        sv = stpool.tile([1, BW], F32, tag="sv")
        # sv = sqrt(vr + D^2*eps) = D*sqrt(var+eps)
        nc.scalar.activation(sv[:], vr[:], AF.Sqrt,
                             bias=eps_t[0:1, 0:1], scale=1.0)
        isv = stpool.tile([1, BW], F32, tag="isv")
        nc.vector.reciprocal(isv[:], sv[:])
        rrow = stpool.tile([1, BW], F32R, tag="rrow")
        nc.vector.tensor_scalar_mul(rrow[:], isv[:], float(D))
        nrm = nrmpool.tile([1, BW], BF16, tag="nrm")
        # nrm = -S1/sv  (rank-1 mean-correction coefficient row)
        nc.vector.scalar_tensor_tensor(
            out=nrm[:], in0=s1r[:], scalar=-1.0, in1=isv[:],
            op0=mybir.AluOpType.mult, op1=mybir.AluOpType.mult)
        # broadcast R down all 128 partitions (f32r matmul keeps it exact)
        rb = rbps.tile([128, BW], F32, tag="rb")
        nc.tensor.matmul(rb[:], ones_rowf[:], rrow[:], start=True, stop=True)
        return rb, nrm

    # ---------------- Phase 1: folded cosine transform ----------------
    pending = {}
    with tc.tile_pool(name="fft_ps", bufs=4, space="PSUM") as fps:
        for ci, (n0, nw) in enumerate(FCHUNKS):
            for mt in range(DT):
                ps = fps.tile([128, 512], F32, tag="fftps")
                for kt in range(KT):
                    nc.tensor.matmul(
                        ps[:, :nw],
                        wf_s[kt][:, mt * 128:(mt + 1) * 128],
                        cf_s[kt][:, n0:n0 + nw],
                        start=(kt == 0), stop=(kt == KT - 1),
                    )
                nc.scalar.copy(zt_s[mt][:, n0:n0 + nw], ps[:, :nw])
            if ci == 0:
                # block-0 stats only need chunk 0; the r-chain then runs on
                # vector/scalar while the PE finishes chunks 1-2
                st0 = emit_stats(0)
            elif ci == 1:
                pending[0] = st0 + emit_rchain(st0[1], st0[2])

    # ---------------- Phase 2: streamed LN+FFN, software-pipelined ----
    ffps = ctx.enter_context(tc.tile_pool(name="ffps", bufs=2, space="PSUM"))
    ops_ = ctx.enter_context(tc.tile_pool(name="ops", bufs=2, space="PSUM"))

    for b in range(NB):
        u_t, s1, s2, rb, nrm = pending.pop(b)

        if b + 1 < NB:
            stn = emit_stats(b + 1)
            pending[b + 1] = stn + emit_rchain(stn[1], stn[2])

        # uhat = u * R (bf16 out)
        uc_t = []
        for dt_ in range(DT):
            uc = ucpool.tile([128, BW], BF16, tag="uc")
            nc.vector.tensor_mul(uc[:], u_t[dt_][:], rb[:])
            uc_t.append(uc)

        # FFN1 (transposed out): A.T[jf, s] accumulated per 128-feat tile
        ht_t = []
        for ft in range(DT):
            psA = ffps.tile([128, BW], F32, tag="psA")
            for kt in range(DT):
                nc.tensor.matmul(psA[:],
                                 w1_s[kt][:, ft * 128:(ft + 1) * 128],
                                 uc_t[kt][:],
                                 start=(kt == 0), stop=False)
            nc.tensor.matmul(psA[:], wsum1_s[0:1, ft * 128:(ft + 1) * 128],
                             nrm[:], start=False, stop=True)
            ht = htpool.tile([128, BW], BF16, tag="ht")
            nc.scalar.activation(ht[:], psA[:], AF.Gelu,
                                 bias=b1p_s[:, ft:ft + 1], scale=1.0)
            ht_t.append(ht)

        # FFN2: out[s, j2] with H.T tiles stationary; b2 added during the
        # PSUM drain on the vector engine
        for tt in range(BW // 128):
            o_t = opool.tile([128, D], F32, tag="o")
            st = b * (BW // 128) + tt
            for c in range(2):
                psO = ops_.tile([128, 512], F32, tag="psO")
                for ft in range(DT):
                    nc.tensor.matmul(psO[:],
                                     ht_t[ft][:, tt * 128:(tt + 1) * 128],
                                     w2_s[ft][:, c * 512:(c + 1) * 512],
                                     start=(ft == 0), stop=(ft == DT - 1))
                nc.vector.tensor_add(o_t[:, c * 512:(c + 1) * 512], psO[:],
                                     b2rep_s[:, c * 512:(c + 1) * 512])
                nc.sync.dma_start(
                    out[st * 128:(st + 1) * 128, c * 512:(c + 1) * 512],
                    o_t[:, c * 512:(c + 1) * 512])


_NC_CACHE = {}


def _build_nc():
    if "nc" in _NC_CACHE:
        return _NC_CACHE["nc"]
    nc = bacc.Bacc("TRN2", target_bir_lowering=False, debug=False)
    xT = nc.declare_dram_parameter("xT", [D, S], BF16, isOutput=False)
    wf = nc.declare_dram_parameter("wf", [TF, D], BF16, isOutput=False)
    cf = nc.declare_dram_parameter("cf", [TF, SFZ], BF16, isOutput=False)
    w1p = nc.declare_dram_parameter("w1p", [D, D], BF16, isOutput=False)
    w2 = nc.declare_dram_parameter("w2", [D, D], BF16, isOutput=False)
    wsum1r = nc.declare_dram_parameter("wsum1r", [1, D], BF16, isOutput=False)
    b1pc = nc.declare_dram_parameter("b1pc", [128, DT], F32, isOutput=False)
    b2rep = nc.declare_dram_parameter("b2rep", [128, D], F32, isOutput=False)
    onescol = nc.declare_dram_parameter("onescol", [128, 1], BF16,
                                        isOutput=False)
    onesrowf = nc.declare_dram_parameter("onesrowf", [1, 128], F32R,
                                         isOutput=False)
    out = nc.declare_dram_parameter("out", [S, D], F32, isOutput=True)
    with tile.TileContext(nc) as tc:
        with ExitStack() as ctx:
            _emit_kernel(ctx, tc, xT, wf, cf, w1p, w2, wsum1r, b1pc, b2rep,
                         onescol, onesrowf, out)
    nc.compile()
    _NC_CACHE["nc"] = nc
    return nc


def _host_prep(x, ln_g, ln_b, w1, b1, w2, b2):
    """Build per-core and shared device inputs."""
    B = x.shape[0]
    # Folded cosine matrix (bf16)
    tt = np.arange(1025, dtype=np.float64)
    Cf = np.zeros((TF, SFZ), BF16NP)
    Cf[:1025, :1025] = np.cos(
        2.0 * np.pi * np.outer(tt, tt) / S).astype(BF16NP)

    w1p = (w1 * ln_g[:, None]).astype(np.float32)
    b1p = (b1 + ln_b @ w1).astype(np.float32)
    wsum1 = w1p.sum(axis=0, dtype=np.float64).astype(BF16NP).reshape(1, D)
    b1pc = np.ascontiguousarray(b1p.reshape(DT, 128).T)  # [128, 8] f32 cols
    b2rep = np.ascontiguousarray(
        np.broadcast_to(b2.astype(np.float32), (128, D)))

    rev = np.concatenate([[0], np.arange(D - 1, 0, -1)])
    shared = dict(cf=Cf, w1p=w1p.astype(BF16NP),
                  w2=np.ascontiguousarray(w2).astype(BF16NP),
                  wsum1r=wsum1, b1pc=b1pc, b2rep=b2rep,
                  onescol=np.ones((128, 1), BF16NP),
                  onesrowf=np.ones((1, 128), np.float32))

    in_maps = []
    for b in range(B):
        xb = np.asarray(x[b], np.float32)
        w = np.float32(D) * xb[:, rev]
        wf_ = np.zeros((TF, D), np.float32)
        wf_[0] = w[0]
        wf_[1024] = w[1024]
        wf_[1:1024] = w[1:1024] + w[2047:1024:-1]
        xT = np.ascontiguousarray(xb.T).astype(BF16NP)
        in_maps.append(dict(xT=xT, wf=wf_.astype(BF16NP), **shared))
    return in_maps


def _run(inputs, trace=False, trace_kwargs=None):
    x = np.asarray(inputs["x"], np.float32)
    in_maps = _host_prep(
        x,
        np.asarray(inputs["ln_g"], np.float32),
        np.asarray(inputs["ln_b"], np.float32),
        np.asarray(inputs["w1"], np.float32),
        np.asarray(inputs["b1"], np.float32),
        np.asarray(inputs["w2"], np.float32),
        np.asarray(inputs["b2"], np.float32),
    )
    nc = _build_nc()
    res = run_bass_kernel_spmd(nc, in_maps, list(range(NCORES)), trace=trace,
                               **(trace_kwargs or {}))
    outs = np.stack([np.asarray(res.results[b]["out"], np.float32)
                     for b in range(NCORES)])
    outs = outs * np.asarray(inputs["mask"], np.float32)
    return outs, res


def kernel(**inputs) -> np.ndarray:
    out, _ = _run(inputs, trace=False)
    return out
